# revision 35
# baseline (speedup 1.0000x reference)
"""Bass/Tile kernel for the XCA-style attention block (v2).

Per-core program (one batch): x [C, HW] bf16 -> y [C, HW] bf16.

Key structural facts exploited:
  * q is a nearest-upsample of a 48-wide parameter: S = q @ k^T collapses
    to S[c,d] = sum_j q48[c,j] * Ksum[d,j] with Ksum = 48 segment-sums of
    k.  Segments align with NB=2048 blocks (6 per block, lengths
    342/341/341/342/341/341).  So k is never materialized globally, never
    transposed; bn_stats gives (count, mean, count*var) per segment from
    which both Ksum and |k|^2 fall out.
  * v lives in SBUF (cc 0,1) / a small DRAM spill (cc 2); proj is folded
    into the attention matrix (Mb = proj @ blockdiag(attn)) so pass 2 is
    one C x C GEMM against v.
  * dwconv taps are split across PE (diag matmuls), DVE (aligned
    stt chains at 2x/4x) and Pool/GPSIMD (stt chains) per a tunable
    (oc, blk) -> engine map.

Pipeline:
  kv0 = Wkv @ x + kv_b                  (PE GEMM, ACT/DVE psum evac)
  k, v = dwconv3x3(kv0) + dw_b          (PE diag / DVE / Pool chains)
  stats = bn_stats(k per segment)       (DVE) -> Ksum, norms2
  lg = q48 @ (rn * Ksum)^T              (PE transpose + tiny gram)
  attn = softmax(lg); MbT = attn @ projT (small DVE/ACT/PE)
  y = Mb @ v + proj_b                   (PE GEMM from SBUF v)
"""
import contextlib
from contextlib import ExitStack

import numpy as np
import ml_dtypes

import concourse.bass as bass
import concourse.mybir as mybir
import concourse.tile as tile
from concourse import bacc

bf16 = mybir.dt.bfloat16
f32 = mybir.dt.float32
AF = mybir.ActivationFunctionType
ALU = mybir.AluOpType
AX = mybir.AxisListType

C = 384
C2 = 768
HEADS = 8
HD = 48
CC = 3            # 128-chunks for C
OC = 6            # 128-chunks for 2C
W = 128
H = 128
HW = H * W
NB = 2048
NBLK = HW // NB
RB = NB // W      # rows per block (16)
EXT = NB + 2 * W  # ext columns (with halo)
GOFF = 2          # left zero-guard cols (even => 4B-aligned dx=0 taps)
GEXT = EXT + 2 * GOFF
SPB = 6           # segments per block
NSEG = NBLK * SPB  # 48
# segment boundaries within a block (pattern repeats every block)
SEGB = [0, 342, 683, 1024, 1366, 1707, 2048]

TAPS = [(dy, dx) for dy in (-1, 0, 1) for dx in (-1, 0, 1)]
CENTER = 4


def head_pieces():
    """Per head: list of (mc, p0, p1, s0): global channels
    [mc*128+p0, mc*128+p1) == within-head channels [s0, s0+(p1-p0))."""
    out = []
    for h in range(HEADS):
        c0, c1 = h * HD, (h + 1) * HD
        pieces = []
        c = c0
        while c < c1:
            mc = c // 128
            p0 = c - mc * 128
            p1 = min(128, c1 - mc * 128)
            pieces.append((mc, p0, p1, c - c0))
            c = mc * 128 + p1
        out.append(pieces)
    return out


def build(cfg, timing_reps=0):
    """cfg keys:
      dw_modes: list of 6 entries 'd' | 'h' | 's'   (v-ocs may be 'd')
      pool_blocks: dict oc -> set of blk indices whose chain runs on Pool
      evac_dve: fraction knobs: (kv_dve_mod, y_dve_mod) alternation mods
    """
    pe_blocks = {k: set(v) for k, v in (cfg.get("pe_blocks") or {}).items()}
    pool_blocks = {k: set(v) for k, v in (cfg.get("pool_blocks") or {}).items()}
    kv_pat = cfg.get("kv_pat", "A")   # evac engine per chunk: A/D
    y_pat = cfg.get("y_pat", "A")
    dw_pat = cfg.get("dw_pat", "A")
    timing = timing_reps > 0

    nc = bacc.Bacc("TRN2", target_bir_lowering=False)

    # ---- DRAM parameters ----
    if timing:
        tok_d = nc.declare_dram_parameter("tok", [1, 1], f32, isOutput=False)
        toko_d = nc.declare_dram_parameter("tok_out", [1, 1], f32, isOutput=True)
        x_d = nc.dram_tensor("x", [C, HW], bf16)
        y_d = nc.dram_tensor("y", [C, HW], bf16)
    else:
        x_d = nc.declare_dram_parameter("x", [C, HW], bf16, isOutput=False)
        y_d = nc.declare_dram_parameter("y", [C, HW], bf16, isOutput=True)
    wkv_d = nc.declare_dram_parameter("wkv", [128, CC, C2], bf16, isOutput=False)
    dws_d = nc.declare_dram_parameter("dws", [128, OC, 9], f32, isOutput=False)
    dwsn_d = nc.declare_dram_parameter("dwsn", [128, OC, 9], f32, isOutput=False)
    kvb_d = nc.declare_dram_parameter("kvb", [128, OC], f32, isOutput=False)
    dwb_d = nc.declare_dram_parameter("dwb", [128, OC], f32, isOutput=False)

    diag_ocs = [oc for oc in range(OC) if pe_blocks.get(oc)]
    diag_slots = {}
    nslot = 0
    for oc in diag_ocs:
        diag_slots[oc] = {t: nslot + t for t in range(9)}
        nslot += 9
    if nslot:
        diag_d = nc.declare_dram_parameter("diag", [128, nslot, 128], bf16,
                                           isOutput=False)
    projT_d = nc.declare_dram_parameter("projT", [HD, HEADS, C], bf16, isOutput=False)
    projb_d = nc.declare_dram_parameter("projb", [128, CC], f32, isOutput=False)
    tempP_d = nc.declare_dram_parameter("tempP", [128, CC], f32, isOutput=False)
    q48T_d = nc.declare_dram_parameter("q48T", [HD, HEADS, HD], bf16, isOutput=False)
    ident_d = nc.declare_dram_parameter("ident", [128, 128], bf16, isOutput=False)

    debug = cfg.get("debug", False)
    if debug:
        dbg_ksum = nc.declare_dram_parameter("dbg_ksum", [128, CC, NBLK, SPB], f32, isOutput=True)
        dbg_rn = nc.declare_dram_parameter("dbg_rn", [128, CC], f32, isOutput=True)
        dbg_lg = nc.declare_dram_parameter("dbg_lg", [HD, HEADS, HD], f32, isOutput=True)
        dbg_mbt = nc.declare_dram_parameter("dbg_mbt", [128, CC, C], f32, isOutput=True)
        dbg_n2 = nc.declare_dram_parameter("dbg_n2", [128, CC], f32, isOutput=True)
        dbg_mbs = nc.declare_dram_parameter("dbg_mbs", [128, CC, C], f32, isOutput=True)

    v12_dram = nc.dram_tensor("v12_spill", [2, 128, HW], bf16)

    xv = x_d[:, :].rearrange("(cc p) n -> p cc n", p=128)
    yv = y_d[:, :].rearrange("(cc p) n -> p cc n", p=128)

    pieces = head_pieces()

    with tile.TileContext(nc) as tc, ExitStack() as ctx:
        const = ctx.enter_context(tc.tile_pool(name="const", bufs=1))
        wkv = const.tile([128, CC, C2], bf16)
        nc.sync.dma_start(out=wkv, in_=wkv_d[:, :, :])
        dws = const.tile([128, OC, 9], f32)
        nc.sync.dma_start(out=dws, in_=dws_d[:, :, :])
        dwsn = const.tile([128, OC, 9], f32)
        nc.sync.dma_start(out=dwsn, in_=dwsn_d[:, :, :])
        kvb = const.tile([128, OC], f32)
        nc.sync.dma_start(out=kvb, in_=kvb_d[:, :])
        dwb = const.tile([128, OC], f32)
        nc.sync.dma_start(out=dwb, in_=dwb_d[:, :])
        if nslot:
            diag = const.tile([128, nslot, 128], bf16)
            nc.sync.dma_start(out=diag, in_=diag_d[:, :, :])
        projT = const.tile([HD, HEADS, C], bf16)
        nc.sync.dma_start(out=projT, in_=projT_d[:, :, :])
        projb = const.tile([128, CC], f32)
        nc.sync.dma_start(out=projb, in_=projb_d[:, :])
        tempP = const.tile([128, CC], f32)
        nc.sync.dma_start(out=tempP, in_=tempP_d[:, :])
        q48T = const.tile([HD, HEADS, HD], bf16)
        nc.sync.dma_start(out=q48T, in_=q48T_d[:, :, :])
        ident = const.tile([128, 128], bf16)
        nc.sync.dma_start(out=ident, in_=ident_d[:, :])

        # persistent state
        v0s = const.tile([128, HW], bf16)               # v for cc 0
        ksum = const.tile([128, CC, NBLK, SPB], f32)
        tmps_po = const.tile([128, 3, CC, NBLK, SPB], f32)
        statsT = const.tile([128, CC, NBLK, SPB, 6], f32)
        MbT = const.tile([128, CC, C], bf16)

        if timing:
            tokt = const.tile([1, 1], f32)
            nc.sync.dma_start(out=tokt, in_=tok_d[:, :])
            nc.sync.dma_start(out=toko_d[:, :], in_=tokt)
            loop_cm = tc.For_i(0, timing_reps, 1)
        else:
            loop_cm = contextlib.nullcontext()

        with loop_cm, ExitStack() as lctx:
            p1 = lctx.enter_context(ExitStack())
            xext = p1.enter_context(tc.tile_pool(name="xext", bufs=2))
            kv0p = p1.enter_context(tc.tile_pool(name="kv0", bufs=2))
            kscrp = p1.enter_context(tc.tile_pool(name="kscr", bufs=2))
            v2sp = p1.enter_context(tc.tile_pool(name="v2s", bufs=2))
            kvmp = p1.enter_context(tc.tile_pool(name="kvm", bufs=1))
            tmpp = p1.enter_context(tc.tile_pool(name="tmp", bufs=3))
            psc = p1.enter_context(tc.tile_pool(name="psc", bufs=2, space="PSUM"))
            psd = p1.enter_context(tc.tile_pool(name="psd", bufs=2, space="PSUM"))

            def dw_pe_taps(oc, kv0, ps, cs):
                """9 taps via PE diag matmuls into psum chunk ps covering
                block cols [cs, cs+1024); tap-outer within the chunk."""
                slots = diag_slots[oc]
                for t, (dy, dx) in enumerate(TAPS):
                    base = GOFF + W + cs + dy * W + dx
                    for h0 in (0, 512):
                        nc.tensor.matmul(
                            ps[:, h0:h0 + 512], lhsT=diag[:, slots[t], :],
                            rhs=kv0[:, 0, base + h0:base + h0 + 512],
                            start=(t == 0), stop=(t == 8))

            def wrap_fixups(oc, kv0, dst, eng):
                """Subtract row-wrap contamination of flat dx-taps at w=0 /
                w=W-1 columns of the SBUF dst."""
                for t, (dy, dx) in enumerate(TAPS):
                    if dx == 0:
                        continue
                    sc = dwsn[:, oc, t:t + 1]
                    if dx == 1:
                        srcc = kv0[:, 0, GOFF + (dy + 2) * W::W][:, :RB]
                        dstc = dst[:, W - 1::W]
                    else:
                        srcc = kv0[:, 0, GOFF - 1 + (dy + 1) * W::W][:, :RB]
                        dstc = dst[:, 0::W]
                    eng.scalar_tensor_tensor(
                        dstc, srcc, sc, dstc, ALU.mult, ALU.add)

            def dw_chain(oc, kv0, dst, tt_pool):
                """dwconv as TS multiplies (DVE 4x via aligned kvm copy) +
                TT adds on DVE (2x) or Pool.  STT is 1x-only on this DVE --
                TS+TT pairs are strictly faster."""
                kvm = kvmp.tile([128, GEXT], bf16, tag="kvm")
                nc.vector.tensor_copy(kvm[:, 0:GEXT - 2], kv0[:, 0, 1:GEXT - 1])
                nc.vector.tensor_scalar(
                    dst, kv0[:, 0, GOFF + W:GOFF + W + NB],
                    dws[:, oc, CENTER:CENTER + 1], dwb[:, oc:oc + 1],
                    ALU.mult, ALU.add)
                tte = nc.gpsimd if tt_pool else nc.vector
                for t, (dy, dx) in enumerate(TAPS):
                    if t == CENTER:
                        continue
                    sc = dws[:, oc, t:t + 1]
                    if dx == 0:
                        srcf = kv0[:, 0, GOFF + (1 + dy) * W:
                                   GOFF + (1 + dy) * W + NB]
                    elif dx == 1:
                        srcf = kvm[:, GOFF + (1 + dy) * W:
                                   GOFF + (1 + dy) * W + NB]
                    else:
                        srcf = kvm[:, GOFF + (1 + dy) * W - 2:
                                   GOFF + (1 + dy) * W - 2 + NB]
                    tmp = tmpp.tile([128, NB], bf16, tag="tmp")
                    nc.vector.tensor_scalar(tmp, srcf, sc, None, ALU.mult)
                    tte.tensor_tensor(dst, dst, tmp, ALU.add)
                wrap_fixups(oc, kv0, dst, nc.vector)

            dwecnt = [0]

            def dw_diag(oc, kv0, dst):
                """All 9 taps on PE diag matmuls; ACT/DVE evac (+bias) to
                SBUF dst; cheap SBUF wrap fixups on DVE."""
                for nch in range(NB // 1024):
                    cs = nch * 1024
                    ps = psd.tile([128, 1024], f32, tag="psd")
                    dw_pe_taps(oc, kv0, ps, cs)
                    dsl = dst[:, cs:cs + 1024]
                    e = dw_pat[dwecnt[0] % len(dw_pat)]
                    dwecnt[0] += 1
                    if e == "A":
                        nc.scalar.activation(dsl, ps, AF.Identity,
                                             bias=dwb[:, oc:oc + 1])
                    else:
                        nc.vector.tensor_scalar_add(dsl, ps, dwb[:, oc:oc + 1])
                wrap_fixups(oc, kv0, dst, nc.vector)

            # flat (blk, oc) pipeline, lag 6: kvconv computes only the NB
            # center columns once; halos are copied from neighbor tiles
            # (dwconv of step j runs after kvconv of step j+6 = next block,
            # same oc, so the right halo source exists).
            OCORD = [3, 4, 5, 0, 1, 2]
            cgrp = [(0, 1024), (1024, 1024)]
            ecnt = 0
            steps = [(blk, oc) for blk in range(NBLK) for oc in OCORD]
            kdsts = {}           # blk -> {cc: tile}
            vsts = {}            # (blk, oc) -> staging tile
            kv0s = {}            # (blk, oc) -> tile
            HB = GOFF + W        # halo+guard copy width (130)

            def emit_kvconv(blk, oc):
                nonlocal ecnt
                n0 = blk * NB
                xe = xes[blk]
                kv0 = kv0p.tile([128, 1, GEXT], bf16, tag=f"kv0_{oc}",
                                name=f"kv0_{oc}")
                kv0s[(blk, oc)] = kv0
                for (cs, cl) in cgrp:
                    ps = psc.tile([128, 1024], f32, tag="psc")
                    for cc in range(CC):
                        for h0 in range(0, cl, 512):
                            nc.tensor.matmul(
                                ps[:, h0:h0 + 512],
                                lhsT=wkv[:, cc, oc * 128:(oc + 1) * 128],
                                rhs=xe[:, cc, cs + h0:cs + h0 + 512],
                                start=(cc == 0), stop=(cc == CC - 1))
                    dst = kv0[:, 0, HB + cs:HB + cs + cl]
                    e = kv_pat[ecnt % len(kv_pat)]
                    ecnt += 1
                    if e == "D":
                        nc.vector.tensor_scalar_add(dst, ps[:, :cl],
                                                    kvb[:, oc:oc + 1])
                    else:
                        nc.scalar.activation(dst, ps[:, :cl], AF.Identity,
                                             bias=kvb[:, oc:oc + 1])
                # left halo+guard [0, HB) from prev tile's last center cols
                if blk == 0:
                    nc.gpsimd.memset(kv0[:, :, 0:HB], 0.0)
                else:
                    prev = kv0s[(blk - 1, oc)]
                    nc.vector.tensor_copy(kv0[:, 0, 0:HB],
                                          prev[:, 0, NB:NB + HB])
                return kv0

            def emit_dwconv(blk, oc):
                n0 = blk * NB
                kv0 = kv0s[(blk, oc)]
                # right halo+guard from next tile's first center cols
                if blk == NBLK - 1:
                    nc.gpsimd.memset(kv0[:, :, HB + NB:], 0.0)
                else:
                    nxt = kv0s[(blk + 1, oc)]
                    nc.vector.tensor_copy(kv0[:, 0, HB + NB:HB + NB + HB],
                                          nxt[:, 0, HB:HB + HB])
                if oc < CC:
                    dst = kscrp.tile([128, NB], bf16, tag=f"k{oc}",
                                     name=f"kscr{oc}")
                    kdsts.setdefault(blk, {})[oc] = dst
                elif oc == CC:
                    dst = v0s[:, n0:n0 + NB]
                else:
                    dst = v2sp.tile([128, NB], bf16, tag=f"v{oc}s",
                                    name=f"vst{oc}")
                    vsts[(blk, oc)] = dst
                if blk in pe_blocks.get(oc, ()):
                    dw_diag(oc, kv0, dst)
                else:
                    dw_chain(oc, kv0, dst,
                             tt_pool=blk in pool_blocks.get(oc, ()))
                kv0s.pop((blk - 1, oc), None)
                if (blk, oc) in vsts:
                    nc.sync.dma_start(
                        out=v12_dram[oc - CC - 1, :, n0:n0 + NB],
                        in_=vsts.pop((blk, oc)))
                if oc == OCORD[-1]:
                    emit_stats(blk)

            def emit_stats(blk):
                kdst = kdsts.pop(blk)
                for cc in range(CC):
                    for s in range(SPB):
                        nc.vector.bn_stats(
                            statsT[:, cc, blk, s, :],
                            kdst[cc][:, SEGB[s]:SEGB[s + 1]])
                stb = statsT[:, :, blk, :, :]
                t1b = tmps_po[:, 0, :, blk, :]
                t2b = tmps_po[:, 1, :, blk, :]
                ksb_sl = ksum[:, :, blk, :]
                nc.vector.tensor_tensor(t1b, stb[:, :, :, 0], stb[:, :, :, 1], ALU.mult)
                nc.vector.tensor_tensor(t2b, stb[:, :, :, 3], stb[:, :, :, 4], ALU.mult)
                nc.vector.tensor_tensor(ksb_sl, t1b, t2b, ALU.add)
                u1 = tmps_po[:, 2, :, blk, :]
                nc.vector.tensor_tensor(u1, t1b, stb[:, :, :, 1], ALU.mult)
                nc.vector.tensor_tensor(t2b, t2b, stb[:, :, :, 4], ALU.mult)
                nc.vector.tensor_tensor(u1, u1, t2b, ALU.add)
                nc.vector.tensor_tensor(u1, u1, stb[:, :, :, 2], ALU.add)
                nc.vector.tensor_tensor(u1, u1, stb[:, :, :, 5], ALU.add)

            xes = {}

            def load_xe(blk):
                n0 = blk * NB
                xe = xext.tile([128, CC, NB], bf16)
                nc.gpsimd.dma_start(out=xe, in_=xv[:, :, n0:n0 + NB])
                xes[blk] = xe

            load_xe(0)
            load_xe(1)
            for j, (blk, oc) in enumerate(steps):
                emit_kvconv(blk, oc)
                if oc == OCORD[0] and blk + 2 < NBLK:
                    load_xe(blk + 2)
                if j >= 6:
                    emit_dwconv(*steps[j - 6])
            for j in range(len(steps) - 6, len(steps)):
                emit_dwconv(*steps[j])

            p1.close()

            # ---------- middle ----------
            pmid = lctx.enter_context(ExitStack())
            mid = pmid.enter_context(tc.tile_pool(name="mid", bufs=1))
            psm = pmid.enter_context(tc.tile_pool(name="psm", bufs=1, space="PSUM"))
            pst = pmid.enter_context(tc.tile_pool(name="pst", bufs=1, space="PSUM"))

            n2 = mid.tile([128, CC], f32)
            nc.vector.tensor_reduce(
                n2, tmps_po[:, 2, :, :, :], axis=AX.XY, op=ALU.add)
            # rnorm = temp / max(sqrt(n2), eps), one Newton step
            sn = mid.tile([128, CC], f32)
            nc.scalar.activation(sn, n2, AF.Sqrt)
            nc.vector.tensor_scalar_max(sn, sn, 1e-12)
            rn = mid.tile([128, CC], f32)
            nc.vector.reciprocal(rn, sn)
            t1 = mid.tile([128, CC], f32)
            nc.vector.tensor_tensor(t1, rn, rn, ALU.mult)
            nc.vector.tensor_tensor(t1, t1, n2, ALU.mult)
            nc.vector.tensor_scalar(t1, t1, -0.5, 1.5, ALU.mult, ALU.add)
            nc.vector.tensor_tensor(rn, rn, t1, ALU.mult)
            nc.vector.tensor_tensor(rn, rn, tempP, ALU.mult)
            # ksb = bf16(rn * ksum)
            ksb = mid.tile([128, CC, NSEG], bf16)
            for cc in range(CC):
                nc.vector.tensor_scalar(
                    ksb[:, cc, :],
                    ksum[:, cc, :, :].rearrange("p b s -> p (b s)"),
                    rn[:, cc:cc + 1], None, ALU.mult)
            # KsT[j, d] via PE transpose
            psT = pst.tile([48, CC, 128], bf16)
            for cc in range(CC):
                nc.tensor.transpose(psT[:, cc, :], ksb[:, cc, :], ident)
            KsT = mid.tile([48, CC, 128], bf16)
            nc.vector.tensor_copy(KsT, psT)
            # lg[c, h, d] = sum_j q48T[j, h, c] * KsT[j, d]
            lgp = pst.tile([HD, HEADS, HD], f32, tag="lgp")
            for h in range(HEADS):
                for (mc, p0, p1_, s0) in pieces[h]:
                    nc.tensor.matmul(
                        lgp[:, h, s0:s0 + (p1_ - p0)],
                        lhsT=q48T[:, h, :], rhs=KsT[:, mc, p0:p1_],
                        start=True, stop=True)
            lg = mid.tile([HD, HEADS, HD], f32)
            nc.vector.tensor_copy(lg, lgp)
            lg2 = lg.rearrange("p h d -> p (h d)")
            mx = mid.tile([HD, HEADS], f32)
            nc.vector.tensor_reduce(mx, lg, axis=AX.X, op=ALU.max)
            nc.vector.tensor_tensor(
                lg, lg, mx[:, :, None].broadcast_to([HD, HEADS, HD]), ALU.subtract)
            nc.scalar.activation(lg2, lg2, AF.Exp)
            sm = mid.tile([HD, HEADS], f32)
            nc.vector.tensor_reduce(sm, lg, axis=AX.X, op=ALU.add)
            nc.vector.reciprocal(sm, sm)
            nc.vector.tensor_tensor(
                lg, lg, sm[:, :, None].broadcast_to([HD, HEADS, HD]), ALU.mult)
            attnb = mid.tile([HD, HEADS, HD], bf16)
            nc.vector.tensor_copy(attnb, lg)

            # Mb[co, d] per mc chunk: pmb[:, mc, h, :] = projT[:,h,mc]^T @ attn
            # one PSUM bank (512 f32) per mc so no (mc, h) slice straddles
            pmb = psm.tile([128, CC, 512], f32)
            for h in range(HEADS):
                for mc in range(CC):
                    nc.tensor.matmul(pmb[:, mc, h * HD:(h + 1) * HD],
                                     lhsT=projT[:, h, mc * 128:(mc + 1) * 128],
                                     rhs=attnb[:, h, :], start=True, stop=True)
            Mb_s = mid.tile([128, CC, C], bf16)
            nc.scalar.activation(Mb_s, pmb[:, :, :C], AF.Identity)
            psMbT = psm.tile([128, CC, C], bf16, tag="psMbT", name="psMbT")
            for dc in range(CC):
                for mc in range(CC):
                    nc.tensor.transpose(
                        psMbT[:, dc, mc * 128:(mc + 1) * 128],
                        Mb_s[:, mc, dc * 128:(dc + 1) * 128], ident)
            nc.vector.tensor_copy(MbT, psMbT)
            if debug:
                nc.sync.dma_start(out=dbg_ksum[:, :, :, :], in_=ksum)
                nc.sync.dma_start(out=dbg_rn[:, :], in_=rn)
                dbg_lg_t = mid.tile([HD, HEADS, HD], f32)
                nc.vector.tensor_copy(dbg_lg_t, lg)
                nc.sync.dma_start(out=dbg_lg[:, :, :], in_=dbg_lg_t)
                dbg_mbt_t = mid.tile([128, CC, C], f32)
                nc.vector.tensor_copy(dbg_mbt_t, MbT)
                nc.sync.dma_start(out=dbg_mbt[:, :, :], in_=dbg_mbt_t)
                nc.sync.dma_start(out=dbg_n2[:, :], in_=n2)
                dbg_mbs_t = mid.tile([128, CC, C], f32)
                nc.vector.tensor_copy(dbg_mbs_t, Mb_s)
                nc.sync.dma_start(out=dbg_mbs[:, :, :], in_=dbg_mbs_t)

            pmid.close()

            # ---------- pass 2 ----------
            p2 = lctx.enter_context(ExitStack())
            vbp = p2.enter_context(tc.tile_pool(name="vb2", bufs=2))
            outp = p2.enter_context(tc.tile_pool(name="outp", bufs=3))
            psf = p2.enter_context(tc.tile_pool(name="psf", bufs=2, space="PSUM"))
            yecnt = 0
            for blk in range(NBLK):
                n0 = blk * NB
                vb12 = vbp.tile([128, 2, NB], bf16, tag="vb12", name="vb12")
                nc.sync.dma_start(
                    out=vb12,
                    in_=v12_dram[:, :, n0:n0 + NB].rearrange("s p n -> p s n"))
                for oc in range(CC):
                    ot = outp.tile([128, NB], bf16)
                    ps = psf.tile([128, NB], f32)
                    for dc in range(CC):
                        vsrc = (v0s[:, n0:n0 + NB] if dc == 0
                                else vb12[:, dc - 1, :])
                        for h0 in range(0, NB, 512):
                            nc.tensor.matmul(
                                ps[:, h0:h0 + 512],
                                lhsT=MbT[:, dc, oc * 128:(oc + 1) * 128],
                                rhs=vsrc[:, h0:h0 + 512],
                                start=(dc == 0), stop=(dc == CC - 1))
                    e = y_pat[yecnt % len(y_pat)]
                    yecnt += 1
                    if e == "D":
                        nc.vector.tensor_scalar_add(ot, ps, projb[:, oc:oc + 1])
                    else:
                        nc.scalar.activation(ot, ps, AF.Identity,
                                             bias=projb[:, oc:oc + 1])
                    nc.sync.dma_start(out=yv[:, oc, n0:n0 + NB], in_=ot)
            p2.close()

    nc.compile()
    return nc


def host_prep(inputs, cfg):
    """Full inputs (numpy, reference layout) -> per-core in_maps list."""
    x = np.ascontiguousarray(inputs["x"]).reshape(-1, C, HW)
    B = x.shape[0]
    qp = np.asarray(inputs["q_param"])[0]              # [heads, hd, 48]
    temp = np.asarray(inputs["temperature"]).reshape(HEADS)
    kv_w = np.asarray(inputs["kv_w"])[:, :, 0, 0]      # [768, 384]
    kv_b = np.asarray(inputs["kv_b"])
    dw_w = np.asarray(inputs["dw_w"])[:, 0]            # [768, 3, 3]
    dw_b = np.asarray(inputs["dw_b"])
    pw = np.asarray(inputs["proj_w"])[:, :, 0, 0]      # [384, 384]
    pb = np.asarray(inputs["proj_b"])

    wkv = np.ascontiguousarray(
        kv_w.T.reshape(CC, 128, C2).transpose(1, 0, 2)).astype(ml_dtypes.bfloat16)
    dws = np.ascontiguousarray(
        dw_w.reshape(OC, 128, 9).transpose(1, 0, 2)).astype(np.float32)
    kvb = np.ascontiguousarray(kv_b.reshape(OC, 128).T).astype(np.float32)
    dwb = np.ascontiguousarray(dw_b.reshape(OC, 128).T).astype(np.float32)

    pe_blocks = cfg.get("pe_blocks") or {}
    slot_list = []
    for oc in range(OC):
        if not pe_blocks.get(oc):
            continue
        for t in range(9):
            slot_list.append((oc, t))
    diag = np.zeros((128, max(len(slot_list), 1), 128), np.float32)
    for i, (oc, t) in enumerate(slot_list):
        s = dw_w[oc * 128:(oc + 1) * 128, t // 3, t % 3]
        diag[np.arange(128), i, np.arange(128)] = s
    diag = diag.astype(ml_dtypes.bfloat16)

    projT = np.ascontiguousarray(
        pw.T.reshape(HEADS, HD, C).transpose(1, 0, 2)).astype(ml_dtypes.bfloat16)
    projb = np.ascontiguousarray(pb.reshape(CC, 128).T).astype(np.float32)
    tempP = np.zeros((128, CC), np.float32)
    for cc in range(CC):
        for p in range(128):
            tempP[p, cc] = temp[(cc * 128 + p) // HD]
    # q48T[j, h, c] = qp[h, c, j]
    q48T = np.ascontiguousarray(qp.transpose(2, 0, 1)).astype(ml_dtypes.bfloat16)
    ident = np.eye(128, dtype=np.float32).astype(ml_dtypes.bfloat16)

    shared = dict(wkv=wkv, dws=dws, dwsn=(-dws).astype(np.float32),
                  kvb=kvb, dwb=dwb, projT=projT, projb=projb, tempP=tempP,
                  q48T=q48T, ident=ident)
    if slot_list:
        shared["diag"] = diag
    in_maps = []
    for b in range(B):
        m = dict(shared)
        m["x"] = x[b].astype(ml_dtypes.bfloat16)
        in_maps.append(m)
    return in_maps


# ---------------------------------------------------------------------------
# Harness entry point: kernel(**inputs) -> full output (B, C, H, W) float32.
# ---------------------------------------------------------------------------

def _carve(ndve, npool):
    pe = {oc: set(range(8)) for oc in range(6)}
    pool = {}
    blkorder = [0, 2, 4, 1, 3, 5, 6, 7]
    rr = []
    for depth, oc in enumerate((5, 4, 3, 2, 1, 0)):
        for b in blkorder:
            rr.append((oc, b))
    for oc, b in rr[:ndve]:
        pe[oc].discard(b)
    for oc, b in rr[ndve:ndve + npool]:
        pe[oc].discard(b)
        pool.setdefault(oc, set()).add(b)
    return pe, pool


_PE_BLOCKS, _POOL_BLOCKS = _carve(18, 0)
CFG = dict(
    pe_blocks=_PE_BLOCKS,
    pool_blocks=_POOL_BLOCKS,
    kv_pat="A",
    y_pat="A",
)

_PROGRAM_CACHE = {}


def _get_program():
    key = "main"
    if key not in _PROGRAM_CACHE:
        _PROGRAM_CACHE[key] = build(CFG)
    return _PROGRAM_CACHE[key]


def kernel(**inputs):
    from concourse.bass_utils import run_bass_kernel_spmd

    x = np.asarray(inputs["x"])
    B, Cin, H_, W_ = x.shape
    assert (Cin, H_, W_) == (C, H, W) and B == 8
    nc = _get_program()
    in_maps = host_prep(inputs, CFG)
    res = run_bass_kernel_spmd(nc, in_maps, list(range(8)))
    out = np.stack([np.asarray(res.results[b]["y"]).astype(np.float32)
                    .reshape(C, H_, W_) for b in range(B)])
    return out


# revision 36
# speedup vs baseline: 1.0802x; 1.0802x over previous
"""Bass/Tile kernel for the XCA-style attention block (v2).

Per-core program (one batch): x [C, HW] bf16 -> y [C, HW] bf16.

Key structural facts exploited:
  * q is a nearest-upsample of a 48-wide parameter: S = q @ k^T collapses
    to S[c,d] = sum_j q48[c,j] * Ksum[d,j] with Ksum = 48 segment-sums of
    k.  Segments align with NB=2048 blocks (6 per block, lengths
    342/341/341/342/341/341).  So k is never materialized globally, never
    transposed; bn_stats gives (count, mean, count*var) per segment from
    which both Ksum and |k|^2 fall out.
  * v lives in SBUF (cc 0,1) / a small DRAM spill (cc 2); proj is folded
    into the attention matrix (Mb = proj @ blockdiag(attn)) so pass 2 is
    one C x C GEMM against v.
  * dwconv taps are split across PE (diag matmuls), DVE (aligned
    stt chains at 2x/4x) and Pool/GPSIMD (stt chains) per a tunable
    (oc, blk) -> engine map.

Pipeline:
  kv0 = Wkv @ x + kv_b                  (PE GEMM, ACT/DVE psum evac)
  k, v = dwconv3x3(kv0) + dw_b          (PE diag / DVE / Pool chains)
  stats = bn_stats(k per segment)       (DVE) -> Ksum, norms2
  lg = q48 @ (rn * Ksum)^T              (PE transpose + tiny gram)
  attn = softmax(lg); MbT = attn @ projT (small DVE/ACT/PE)
  y = Mb @ v + proj_b                   (PE GEMM from SBUF v)
"""
import contextlib
from contextlib import ExitStack

import numpy as np
import ml_dtypes

import concourse.bass as bass
import concourse.mybir as mybir
import concourse.tile as tile
from concourse import bacc

bf16 = mybir.dt.bfloat16
f32 = mybir.dt.float32
AF = mybir.ActivationFunctionType
ALU = mybir.AluOpType
AX = mybir.AxisListType

C = 384
C2 = 768
HEADS = 8
HD = 48
CC = 3            # 128-chunks for C
OC = 6            # 128-chunks for 2C
W = 128
H = 128
HW = H * W
NB = 2048
NBLK = HW // NB
RB = NB // W      # rows per block (16)
EXT = NB + 2 * W  # ext columns (with halo)
GOFF = 2          # left zero-guard cols (even => 4B-aligned dx=0 taps)
GEXT = EXT + 2 * GOFF
SPB = 6           # segments per block
NSEG = NBLK * SPB  # 48
# segment boundaries within a block (pattern repeats every block)
SEGB = [0, 342, 683, 1024, 1366, 1707, 2048]

TAPS = [(dy, dx) for dy in (-1, 0, 1) for dx in (-1, 0, 1)]
CENTER = 4


def head_pieces():
    """Per head: list of (mc, p0, p1, s0): global channels
    [mc*128+p0, mc*128+p1) == within-head channels [s0, s0+(p1-p0))."""
    out = []
    for h in range(HEADS):
        c0, c1 = h * HD, (h + 1) * HD
        pieces = []
        c = c0
        while c < c1:
            mc = c // 128
            p0 = c - mc * 128
            p1 = min(128, c1 - mc * 128)
            pieces.append((mc, p0, p1, c - c0))
            c = mc * 128 + p1
        out.append(pieces)
    return out


def build(cfg, timing_reps=0):
    """cfg keys:
      dw_modes: list of 6 entries 'd' | 'h' | 's'   (v-ocs may be 'd')
      pool_blocks: dict oc -> set of blk indices whose chain runs on Pool
      evac_dve: fraction knobs: (kv_dve_mod, y_dve_mod) alternation mods
    """
    pe_blocks = {k: set(v) for k, v in (cfg.get("pe_blocks") or {}).items()}
    pool_blocks = {k: set(v) for k, v in (cfg.get("pool_blocks") or {}).items()}
    kv_pat = cfg.get("kv_pat", "A")   # evac engine per chunk: A/D
    y_pat = cfg.get("y_pat", "A")
    dw_pat = cfg.get("dw_pat", "A")
    timing = timing_reps > 0

    nc = bacc.Bacc("TRN2", target_bir_lowering=False)

    # ---- DRAM parameters ----
    if timing:
        tok_d = nc.declare_dram_parameter("tok", [1, 1], f32, isOutput=False)
        toko_d = nc.declare_dram_parameter("tok_out", [1, 1], f32, isOutput=True)
        x_d = nc.dram_tensor("x", [C, HW], bf16)
        y_d = nc.dram_tensor("y", [C, HW], bf16)
    else:
        x_d = nc.declare_dram_parameter("x", [C, HW], bf16, isOutput=False)
        y_d = nc.declare_dram_parameter("y", [C, HW], bf16, isOutput=True)
    wkv_d = nc.declare_dram_parameter("wkv", [128, CC, C2], bf16, isOutput=False)
    dws_d = nc.declare_dram_parameter("dws", [128, OC, 9], f32, isOutput=False)
    dwsn_d = nc.declare_dram_parameter("dwsn", [128, OC, 9], f32, isOutput=False)
    kvb_d = nc.declare_dram_parameter("kvb", [128, OC], f32, isOutput=False)
    dwb_d = nc.declare_dram_parameter("dwb", [128, OC], f32, isOutput=False)

    diag_ocs = [oc for oc in range(OC) if pe_blocks.get(oc)]
    diag_slots = {}
    nslot = 0
    for oc in diag_ocs:
        diag_slots[oc] = {t: nslot + t for t in range(9)}
        nslot += 9
    if nslot:
        diag_d = nc.declare_dram_parameter("diag", [128, nslot, 128], bf16,
                                           isOutput=False)
    projT_d = nc.declare_dram_parameter("projT", [HD, HEADS, C], bf16, isOutput=False)
    projb_d = nc.declare_dram_parameter("projb", [128, CC], f32, isOutput=False)
    tempP_d = nc.declare_dram_parameter("tempP", [128, CC], f32, isOutput=False)
    q48T_d = nc.declare_dram_parameter("q48T", [HD, HEADS, HD], bf16, isOutput=False)
    ident_d = nc.declare_dram_parameter("ident", [128, 128], bf16, isOutput=False)

    debug = cfg.get("debug", False)
    if debug:
        dbg_ksum = nc.declare_dram_parameter("dbg_ksum", [128, CC, NBLK, SPB], f32, isOutput=True)
        dbg_rn = nc.declare_dram_parameter("dbg_rn", [128, CC], f32, isOutput=True)
        dbg_lg = nc.declare_dram_parameter("dbg_lg", [HD, HEADS, HD], f32, isOutput=True)
        dbg_mbt = nc.declare_dram_parameter("dbg_mbt", [128, CC, C], f32, isOutput=True)
        dbg_n2 = nc.declare_dram_parameter("dbg_n2", [128, CC], f32, isOutput=True)
        dbg_mbs = nc.declare_dram_parameter("dbg_mbs", [128, CC, C], f32, isOutput=True)

    v12_dram = nc.dram_tensor("v12_spill", [2, 128, HW], bf16)

    xv = x_d[:, :].rearrange("(cc p) n -> p cc n", p=128)
    yv = y_d[:, :].rearrange("(cc p) n -> p cc n", p=128)

    pieces = head_pieces()

    with tile.TileContext(nc) as tc, ExitStack() as ctx:
        const = ctx.enter_context(tc.tile_pool(name="const", bufs=1))
        wkv = const.tile([128, CC, C2], bf16)
        nc.sync.dma_start(out=wkv, in_=wkv_d[:, :, :])
        dws = const.tile([128, OC, 9], f32)
        nc.sync.dma_start(out=dws, in_=dws_d[:, :, :])
        dwsn = const.tile([128, OC, 9], f32)
        nc.sync.dma_start(out=dwsn, in_=dwsn_d[:, :, :])
        kvb = const.tile([128, OC], f32)
        nc.sync.dma_start(out=kvb, in_=kvb_d[:, :])
        dwb = const.tile([128, OC], f32)
        nc.sync.dma_start(out=dwb, in_=dwb_d[:, :])
        if nslot:
            diag = const.tile([128, nslot, 128], bf16)
            nc.sync.dma_start(out=diag, in_=diag_d[:, :, :])
        projT = const.tile([HD, HEADS, C], bf16)
        nc.sync.dma_start(out=projT, in_=projT_d[:, :, :])
        projb = const.tile([128, CC], f32)
        nc.sync.dma_start(out=projb, in_=projb_d[:, :])
        tempP = const.tile([128, CC], f32)
        nc.sync.dma_start(out=tempP, in_=tempP_d[:, :])
        q48T = const.tile([HD, HEADS, HD], bf16)
        nc.sync.dma_start(out=q48T, in_=q48T_d[:, :, :])
        ident = const.tile([128, 128], bf16)
        nc.sync.dma_start(out=ident, in_=ident_d[:, :])

        # persistent state
        v0s = const.tile([128, HW], bf16)               # v for cc 0
        ksum = const.tile([128, CC, NBLK, SPB], f32)
        tmps_po = const.tile([128, 3, CC, NBLK, SPB], f32)
        statsT = const.tile([128, CC, NBLK, SPB, 6], f32)
        MbT = const.tile([128, CC, C], bf16)

        if timing:
            tokt = const.tile([1, 1], f32)
            nc.sync.dma_start(out=tokt, in_=tok_d[:, :])
            nc.sync.dma_start(out=toko_d[:, :], in_=tokt)
            loop_cm = tc.For_i(0, timing_reps, 1)
        else:
            loop_cm = contextlib.nullcontext()

        with loop_cm, ExitStack() as lctx:
            p1 = lctx.enter_context(ExitStack())
            xext = p1.enter_context(tc.tile_pool(name="xext", bufs=2))
            kv0p = p1.enter_context(tc.tile_pool(name="kv0", bufs=2))
            kscrp = p1.enter_context(tc.tile_pool(name="kscr", bufs=2))
            v2sp = p1.enter_context(tc.tile_pool(name="v2s", bufs=2))
            kvmp = p1.enter_context(tc.tile_pool(name="kvm", bufs=1))
            tmpp = p1.enter_context(tc.tile_pool(name="tmp", bufs=3))
            psc = p1.enter_context(tc.tile_pool(name="psc", bufs=2, space="PSUM"))
            psd = p1.enter_context(tc.tile_pool(name="psd", bufs=2, space="PSUM"))

            def dw_pe_taps(oc, kv0, ps, cs):
                """9 taps via PE diag matmuls into psum chunk ps covering
                block cols [cs, cs+1024); tap-outer within the chunk."""
                slots = diag_slots[oc]
                for t, (dy, dx) in enumerate(TAPS):
                    base = GOFF + W + cs + dy * W + dx
                    for h0 in (0, 512):
                        nc.tensor.matmul(
                            ps[:, h0:h0 + 512], lhsT=diag[:, slots[t], :],
                            rhs=kv0[:, 0, base + h0:base + h0 + 512],
                            start=(t == 0), stop=(t == 8))

            def wrap_fixups(oc, kv0, dst, eng):
                """Subtract row-wrap contamination of flat dx-taps at w=0 /
                w=W-1 columns of the SBUF dst."""
                for t, (dy, dx) in enumerate(TAPS):
                    if dx == 0:
                        continue
                    sc = dwsn[:, oc, t:t + 1]
                    if dx == 1:
                        srcc = kv0[:, 0, GOFF + (dy + 2) * W::W][:, :RB]
                        dstc = dst[:, W - 1::W]
                    else:
                        srcc = kv0[:, 0, GOFF - 1 + (dy + 1) * W::W][:, :RB]
                        dstc = dst[:, 0::W]
                    eng.scalar_tensor_tensor(
                        dstc, srcc, sc, dstc, ALU.mult, ALU.add)

            def dw_chain(oc, kv0, dst, tt_pool):
                """dwconv as TS multiplies (DVE 4x via aligned kvm copy) +
                TT adds on DVE (2x) or Pool.  STT is 1x-only on this DVE --
                TS+TT pairs are strictly faster."""
                kvm = kvmp.tile([128, GEXT], bf16, tag="kvm")
                nc.vector.tensor_copy(kvm[:, 0:GEXT - 2], kv0[:, 0, 1:GEXT - 1])
                nc.vector.tensor_scalar(
                    dst, kv0[:, 0, GOFF + W:GOFF + W + NB],
                    dws[:, oc, CENTER:CENTER + 1], dwb[:, oc:oc + 1],
                    ALU.mult, ALU.add)
                tte = nc.gpsimd if tt_pool else nc.vector
                for t, (dy, dx) in enumerate(TAPS):
                    if t == CENTER:
                        continue
                    sc = dws[:, oc, t:t + 1]
                    if dx == 0:
                        srcf = kv0[:, 0, GOFF + (1 + dy) * W:
                                   GOFF + (1 + dy) * W + NB]
                    elif dx == 1:
                        srcf = kvm[:, GOFF + (1 + dy) * W:
                                   GOFF + (1 + dy) * W + NB]
                    else:
                        srcf = kvm[:, GOFF + (1 + dy) * W - 2:
                                   GOFF + (1 + dy) * W - 2 + NB]
                    tmp = tmpp.tile([128, NB], bf16, tag="tmp")
                    nc.vector.tensor_scalar(tmp, srcf, sc, None, ALU.mult)
                    tte.tensor_tensor(dst, dst, tmp, ALU.add)
                wrap_fixups(oc, kv0, dst, nc.vector)

            dwecnt = [0]

            def dw_diag(oc, kv0, dst):
                """All 9 taps on PE diag matmuls; ACT/DVE evac (+bias) to
                SBUF dst; cheap SBUF wrap fixups on DVE."""
                for nch in range(NB // 1024):
                    cs = nch * 1024
                    ps = psd.tile([128, 1024], f32, tag="psd")
                    dw_pe_taps(oc, kv0, ps, cs)
                    dsl = dst[:, cs:cs + 1024]
                    e = dw_pat[dwecnt[0] % len(dw_pat)]
                    dwecnt[0] += 1
                    if e == "A":
                        nc.scalar.activation(dsl, ps, AF.Identity,
                                             bias=dwb[:, oc:oc + 1])
                    else:
                        nc.vector.tensor_scalar_add(dsl, ps, dwb[:, oc:oc + 1])
                wrap_fixups(oc, kv0, dst, nc.vector)

            # flat (blk, oc) pipeline: kvconv of step j overlaps dwconv of
            # step j-1 so PE never waits on ACT's kv0 evacuation.
            OCORD = [3, 4, 5, 0, 1, 2]
            cgrp = [(0, 1024), (1024, 1024), (2048, 256)]
            ecnt = 0
            steps = [(blk, oc) for blk in range(NBLK) for oc in OCORD]
            pend = None
            kdsts = {}
            vsts = {}

            def emit_kvconv(blk, oc):
                nonlocal ecnt
                n0 = blk * NB
                lo = max(0, n0 - W)
                hi = min(HW, n0 + NB + W)
                xe = xes[blk]
                kv0 = kv0p.tile([128, 1, GEXT], bf16, tag=f"kv0_{oc}",
                                name=f"kv0_{oc}")
                for (cs, cl) in cgrp:
                    ps = psc.tile([128, 1024], f32, tag="psc")
                    for cc in range(CC):
                        for h0 in range(0, cl, 512):
                            hl = min(512, cl - h0)
                            nc.tensor.matmul(
                                ps[:, h0:h0 + hl],
                                lhsT=wkv[:, cc, oc * 128:(oc + 1) * 128],
                                rhs=xe[:, cc, cs + h0:cs + h0 + hl],
                                start=(cc == 0), stop=(cc == CC - 1))
                    dst = kv0[:, 0, GOFF + cs:GOFF + cs + cl]
                    e = kv_pat[ecnt % len(kv_pat)]
                    ecnt += 1
                    if e == "D":
                        nc.vector.tensor_scalar_add(dst, ps[:, :cl],
                                                    kvb[:, oc:oc + 1])
                    else:
                        nc.scalar.activation(dst, ps[:, :cl], AF.Identity,
                                             bias=kvb[:, oc:oc + 1])
                if lo == 0:
                    nc.gpsimd.memset(kv0[:, :, :GOFF + W], 0.0)
                else:
                    nc.gpsimd.memset(kv0[:, :, 0:GOFF], 0.0)
                if hi == HW:
                    nc.gpsimd.memset(kv0[:, :, GOFF + W + NB:], 0.0)
                else:
                    nc.gpsimd.memset(kv0[:, :, GEXT - GOFF:], 0.0)
                return kv0

            def emit_dwconv(blk, oc, kv0):
                n0 = blk * NB
                if oc < CC:
                    dst = kscrp.tile([128, NB], bf16, tag=f"k{oc}",
                                     name=f"kscr{oc}")
                    kdsts.setdefault(blk, {})[oc] = dst
                elif oc == CC:
                    dst = v0s[:, n0:n0 + NB]
                else:
                    dst = v2sp.tile([128, NB], bf16, tag=f"v{oc}s",
                                    name=f"vst{oc}")
                    vsts[(blk, oc)] = dst
                if blk in pe_blocks.get(oc, ()):
                    dw_diag(oc, kv0, dst)
                else:
                    dw_chain(oc, kv0, dst,
                             tt_pool=blk in pool_blocks.get(oc, ()))
                if (blk, oc) in vsts:
                    nc.sync.dma_start(
                        out=v12_dram[oc - CC - 1, :, n0:n0 + NB],
                        in_=vsts.pop((blk, oc)))
                if oc == OCORD[-1]:
                    emit_stats(blk)

            def emit_stats(blk):
                kdst = kdsts.pop(blk)
                for cc in range(CC):
                    for s in range(SPB):
                        nc.vector.bn_stats(
                            statsT[:, cc, blk, s, :],
                            kdst[cc][:, SEGB[s]:SEGB[s + 1]])
                stb = statsT[:, :, blk, :, :]
                t1b = tmps_po[:, 0, :, blk, :]
                t2b = tmps_po[:, 1, :, blk, :]
                ksb_sl = ksum[:, :, blk, :]
                nc.vector.tensor_tensor(t1b, stb[:, :, :, 0], stb[:, :, :, 1], ALU.mult)
                nc.vector.tensor_tensor(t2b, stb[:, :, :, 3], stb[:, :, :, 4], ALU.mult)
                nc.vector.tensor_tensor(ksb_sl, t1b, t2b, ALU.add)
                u1 = tmps_po[:, 2, :, blk, :]
                nc.vector.tensor_tensor(u1, t1b, stb[:, :, :, 1], ALU.mult)
                nc.vector.tensor_tensor(t2b, t2b, stb[:, :, :, 4], ALU.mult)
                nc.vector.tensor_tensor(u1, u1, t2b, ALU.add)
                nc.vector.tensor_tensor(u1, u1, stb[:, :, :, 2], ALU.add)
                nc.vector.tensor_tensor(u1, u1, stb[:, :, :, 5], ALU.add)

            xes = {}

            def load_xe(blk):
                n0 = blk * NB
                lo = max(0, n0 - W)
                hi = min(HW, n0 + NB + W)
                off = lo - (n0 - W)
                xe = xext.tile([128, CC, EXT], bf16)
                if off > 0:
                    nc.vector.memset(xe[:, :, :off], 0.0)
                if (n0 - W) + EXT > hi:
                    nc.vector.memset(xe[:, :, off + (hi - lo):], 0.0)
                nc.gpsimd.dma_start(out=xe[:, :, off:off + (hi - lo)],
                                    in_=xv[:, :, lo:hi])
                xes[blk] = xe

            load_xe(0)
            load_xe(1)
            for j, (blk, oc) in enumerate(steps):
                kv0 = emit_kvconv(blk, oc)
                if oc == OCORD[0] and blk + 2 < NBLK:
                    load_xe(blk + 2)
                if pend is not None:
                    emit_dwconv(*pend)
                pend = (blk, oc, kv0)
            emit_dwconv(*pend)

            p1.close()

            # ---------- middle ----------
            pmid = lctx.enter_context(ExitStack())
            mid = pmid.enter_context(tc.tile_pool(name="mid", bufs=1))
            psm = pmid.enter_context(tc.tile_pool(name="psm", bufs=1, space="PSUM"))
            pst = pmid.enter_context(tc.tile_pool(name="pst", bufs=1, space="PSUM"))

            n2 = mid.tile([128, CC], f32)
            nc.vector.tensor_reduce(
                n2, tmps_po[:, 2, :, :, :], axis=AX.XY, op=ALU.add)
            # rnorm = temp / max(sqrt(n2), eps), one Newton step
            sn = mid.tile([128, CC], f32)
            nc.scalar.activation(sn, n2, AF.Sqrt)
            nc.vector.tensor_scalar_max(sn, sn, 1e-12)
            rn = mid.tile([128, CC], f32)
            nc.vector.reciprocal(rn, sn)
            t1 = mid.tile([128, CC], f32)
            nc.vector.tensor_tensor(t1, rn, rn, ALU.mult)
            nc.vector.tensor_tensor(t1, t1, n2, ALU.mult)
            nc.vector.tensor_scalar(t1, t1, -0.5, 1.5, ALU.mult, ALU.add)
            nc.vector.tensor_tensor(rn, rn, t1, ALU.mult)
            nc.vector.tensor_tensor(rn, rn, tempP, ALU.mult)
            # ksb = bf16(rn * ksum)
            ksb = mid.tile([128, CC, NSEG], bf16)
            for cc in range(CC):
                nc.vector.tensor_scalar(
                    ksb[:, cc, :],
                    ksum[:, cc, :, :].rearrange("p b s -> p (b s)"),
                    rn[:, cc:cc + 1], None, ALU.mult)
            # KsT[j, d] via PE transpose
            psT = pst.tile([48, CC, 128], bf16)
            for cc in range(CC):
                nc.tensor.transpose(psT[:, cc, :], ksb[:, cc, :], ident)
            KsT = mid.tile([48, CC, 128], bf16)
            nc.vector.tensor_copy(KsT, psT)
            # lg[c, h, d] = sum_j q48T[j, h, c] * KsT[j, d]
            lgp = pst.tile([HD, HEADS, HD], f32, tag="lgp")
            for h in range(HEADS):
                for (mc, p0, p1_, s0) in pieces[h]:
                    nc.tensor.matmul(
                        lgp[:, h, s0:s0 + (p1_ - p0)],
                        lhsT=q48T[:, h, :], rhs=KsT[:, mc, p0:p1_],
                        start=True, stop=True)
            lg = mid.tile([HD, HEADS, HD], f32)
            nc.vector.tensor_copy(lg, lgp)
            lg2 = lg.rearrange("p h d -> p (h d)")
            mx = mid.tile([HD, HEADS], f32)
            nc.vector.tensor_reduce(mx, lg, axis=AX.X, op=ALU.max)
            nc.vector.tensor_tensor(
                lg, lg, mx[:, :, None].broadcast_to([HD, HEADS, HD]), ALU.subtract)
            nc.scalar.activation(lg2, lg2, AF.Exp)
            sm = mid.tile([HD, HEADS], f32)
            nc.vector.tensor_reduce(sm, lg, axis=AX.X, op=ALU.add)
            nc.vector.reciprocal(sm, sm)
            nc.vector.tensor_tensor(
                lg, lg, sm[:, :, None].broadcast_to([HD, HEADS, HD]), ALU.mult)
            attnb = mid.tile([HD, HEADS, HD], bf16)
            nc.vector.tensor_copy(attnb, lg)

            # Mb[co, d] per mc chunk: pmb[:, mc, h, :] = projT[:,h,mc]^T @ attn
            # one PSUM bank (512 f32) per mc so no (mc, h) slice straddles
            pmb = psm.tile([128, CC, 512], f32)
            for h in range(HEADS):
                for mc in range(CC):
                    nc.tensor.matmul(pmb[:, mc, h * HD:(h + 1) * HD],
                                     lhsT=projT[:, h, mc * 128:(mc + 1) * 128],
                                     rhs=attnb[:, h, :], start=True, stop=True)
            Mb_s = mid.tile([128, CC, C], bf16)
            nc.scalar.activation(Mb_s, pmb[:, :, :C], AF.Identity)
            psMbT = psm.tile([128, CC, C], bf16, tag="psMbT", name="psMbT")
            for dc in range(CC):
                for mc in range(CC):
                    nc.tensor.transpose(
                        psMbT[:, dc, mc * 128:(mc + 1) * 128],
                        Mb_s[:, mc, dc * 128:(dc + 1) * 128], ident)
            nc.vector.tensor_copy(MbT, psMbT)
            if debug:
                nc.sync.dma_start(out=dbg_ksum[:, :, :, :], in_=ksum)
                nc.sync.dma_start(out=dbg_rn[:, :], in_=rn)
                dbg_lg_t = mid.tile([HD, HEADS, HD], f32)
                nc.vector.tensor_copy(dbg_lg_t, lg)
                nc.sync.dma_start(out=dbg_lg[:, :, :], in_=dbg_lg_t)
                dbg_mbt_t = mid.tile([128, CC, C], f32)
                nc.vector.tensor_copy(dbg_mbt_t, MbT)
                nc.sync.dma_start(out=dbg_mbt[:, :, :], in_=dbg_mbt_t)
                nc.sync.dma_start(out=dbg_n2[:, :], in_=n2)
                dbg_mbs_t = mid.tile([128, CC, C], f32)
                nc.vector.tensor_copy(dbg_mbs_t, Mb_s)
                nc.sync.dma_start(out=dbg_mbs[:, :, :], in_=dbg_mbs_t)

            pmid.close()

            # ---------- pass 2 ----------
            p2 = lctx.enter_context(ExitStack())
            vbp = p2.enter_context(tc.tile_pool(name="vb2", bufs=2))
            outp = p2.enter_context(tc.tile_pool(name="outp", bufs=3))
            psf = p2.enter_context(tc.tile_pool(name="psf", bufs=2, space="PSUM"))
            yecnt = 0
            for blk in range(NBLK):
                n0 = blk * NB
                vb12 = vbp.tile([128, 2, NB], bf16, tag="vb12", name="vb12")
                nc.sync.dma_start(
                    out=vb12,
                    in_=v12_dram[:, :, n0:n0 + NB].rearrange("s p n -> p s n"))
                for oc in range(CC):
                    ot = outp.tile([128, NB], bf16)
                    ps = psf.tile([128, NB], f32)
                    for dc in range(CC):
                        vsrc = (v0s[:, n0:n0 + NB] if dc == 0
                                else vb12[:, dc - 1, :])
                        for h0 in range(0, NB, 512):
                            nc.tensor.matmul(
                                ps[:, h0:h0 + 512],
                                lhsT=MbT[:, dc, oc * 128:(oc + 1) * 128],
                                rhs=vsrc[:, h0:h0 + 512],
                                start=(dc == 0), stop=(dc == CC - 1))
                    e = y_pat[yecnt % len(y_pat)]
                    yecnt += 1
                    if e == "D":
                        nc.vector.tensor_scalar_add(ot, ps, projb[:, oc:oc + 1])
                    else:
                        nc.scalar.activation(ot, ps, AF.Identity,
                                             bias=projb[:, oc:oc + 1])
                    nc.sync.dma_start(out=yv[:, oc, n0:n0 + NB], in_=ot)
            p2.close()

    nc.compile()
    return nc


def host_prep(inputs, cfg):
    """Full inputs (numpy, reference layout) -> per-core in_maps list."""
    x = np.ascontiguousarray(inputs["x"]).reshape(-1, C, HW)
    B = x.shape[0]
    qp = np.asarray(inputs["q_param"])[0]              # [heads, hd, 48]
    temp = np.asarray(inputs["temperature"]).reshape(HEADS)
    kv_w = np.asarray(inputs["kv_w"])[:, :, 0, 0]      # [768, 384]
    kv_b = np.asarray(inputs["kv_b"])
    dw_w = np.asarray(inputs["dw_w"])[:, 0]            # [768, 3, 3]
    dw_b = np.asarray(inputs["dw_b"])
    pw = np.asarray(inputs["proj_w"])[:, :, 0, 0]      # [384, 384]
    pb = np.asarray(inputs["proj_b"])

    wkv = np.ascontiguousarray(
        kv_w.T.reshape(CC, 128, C2).transpose(1, 0, 2)).astype(ml_dtypes.bfloat16)
    dws = np.ascontiguousarray(
        dw_w.reshape(OC, 128, 9).transpose(1, 0, 2)).astype(np.float32)
    kvb = np.ascontiguousarray(kv_b.reshape(OC, 128).T).astype(np.float32)
    dwb = np.ascontiguousarray(dw_b.reshape(OC, 128).T).astype(np.float32)

    pe_blocks = cfg.get("pe_blocks") or {}
    slot_list = []
    for oc in range(OC):
        if not pe_blocks.get(oc):
            continue
        for t in range(9):
            slot_list.append((oc, t))
    diag = np.zeros((128, max(len(slot_list), 1), 128), np.float32)
    for i, (oc, t) in enumerate(slot_list):
        s = dw_w[oc * 128:(oc + 1) * 128, t // 3, t % 3]
        diag[np.arange(128), i, np.arange(128)] = s
    diag = diag.astype(ml_dtypes.bfloat16)

    projT = np.ascontiguousarray(
        pw.T.reshape(HEADS, HD, C).transpose(1, 0, 2)).astype(ml_dtypes.bfloat16)
    projb = np.ascontiguousarray(pb.reshape(CC, 128).T).astype(np.float32)
    tempP = np.zeros((128, CC), np.float32)
    for cc in range(CC):
        for p in range(128):
            tempP[p, cc] = temp[(cc * 128 + p) // HD]
    # q48T[j, h, c] = qp[h, c, j]
    q48T = np.ascontiguousarray(qp.transpose(2, 0, 1)).astype(ml_dtypes.bfloat16)
    ident = np.eye(128, dtype=np.float32).astype(ml_dtypes.bfloat16)

    shared = dict(wkv=wkv, dws=dws, dwsn=(-dws).astype(np.float32),
                  kvb=kvb, dwb=dwb, projT=projT, projb=projb, tempP=tempP,
                  q48T=q48T, ident=ident)
    if slot_list:
        shared["diag"] = diag
    in_maps = []
    for b in range(B):
        m = dict(shared)
        m["x"] = x[b].astype(ml_dtypes.bfloat16)
        in_maps.append(m)
    return in_maps


# ---------------------------------------------------------------------------
# Harness entry point: kernel(**inputs) -> full output (B, C, H, W) float32.
# ---------------------------------------------------------------------------

def _carve(ndve, npool):
    pe = {oc: set(range(8)) for oc in range(6)}
    pool = {}
    blkorder = [0, 2, 4, 1, 3, 5, 6, 7]
    rr = []
    for depth, oc in enumerate((5, 4, 3, 2, 1, 0)):
        for b in blkorder:
            rr.append((oc, b))
    for oc, b in rr[:ndve]:
        pe[oc].discard(b)
    for oc, b in rr[ndve:ndve + npool]:
        pe[oc].discard(b)
        pool.setdefault(oc, set()).add(b)
    return pe, pool


_PE_BLOCKS, _POOL_BLOCKS = _carve(18, 0)
CFG = dict(
    pe_blocks=_PE_BLOCKS,
    pool_blocks=_POOL_BLOCKS,
    kv_pat="A",
    y_pat="A",
)

_PROGRAM_CACHE = {}


def _get_program():
    key = "main"
    if key not in _PROGRAM_CACHE:
        _PROGRAM_CACHE[key] = build(CFG)
    return _PROGRAM_CACHE[key]


def kernel(**inputs):
    from concourse.bass_utils import run_bass_kernel_spmd

    x = np.asarray(inputs["x"])
    B, Cin, H_, W_ = x.shape
    assert (Cin, H_, W_) == (C, H, W) and B == 8
    nc = _get_program()
    in_maps = host_prep(inputs, CFG)
    res = run_bass_kernel_spmd(nc, in_maps, list(range(8)))
    out = np.stack([np.asarray(res.results[b]["y"]).astype(np.float32)
                    .reshape(C, H_, W_) for b in range(B)])
    return out


# revision 37
# speedup vs baseline: 1.2419x; 1.1496x over previous
"""Bass/Tile kernel for the XCA-style attention block (v2).

Per-core program (one batch): x [C, HW] bf16 -> y [C, HW] bf16.

Key structural facts exploited:
  * q is a nearest-upsample of a 48-wide parameter: S = q @ k^T collapses
    to S[c,d] = sum_j q48[c,j] * Ksum[d,j] with Ksum = 48 segment-sums of
    k.  Segments align with NB=2048 blocks (6 per block, lengths
    342/341/341/342/341/341).  So k is never materialized globally, never
    transposed; bn_stats gives (count, mean, count*var) per segment from
    which both Ksum and |k|^2 fall out.
  * v lives in SBUF (cc 0,1) / a small DRAM spill (cc 2); proj is folded
    into the attention matrix (Mb = proj @ blockdiag(attn)) so pass 2 is
    one C x C GEMM against v.
  * dwconv taps are split across PE (diag matmuls), DVE (aligned
    stt chains at 2x/4x) and Pool/GPSIMD (stt chains) per a tunable
    (oc, blk) -> engine map.

Pipeline:
  kv0 = Wkv @ x + kv_b                  (PE GEMM, ACT/DVE psum evac)
  k, v = dwconv3x3(kv0) + dw_b          (PE diag / DVE / Pool chains)
  stats = bn_stats(k per segment)       (DVE) -> Ksum, norms2
  lg = q48 @ (rn * Ksum)^T              (PE transpose + tiny gram)
  attn = softmax(lg); MbT = attn @ projT (small DVE/ACT/PE)
  y = Mb @ v + proj_b                   (PE GEMM from SBUF v)
"""
import contextlib
from contextlib import ExitStack

import numpy as np
import ml_dtypes

import concourse.bass as bass
import concourse.mybir as mybir
import concourse.tile as tile
from concourse import bacc

bf16 = mybir.dt.bfloat16
f32 = mybir.dt.float32
AF = mybir.ActivationFunctionType
ALU = mybir.AluOpType
AX = mybir.AxisListType

C = 384
C2 = 768
HEADS = 8
HD = 48
CC = 3            # 128-chunks for C
OC = 6            # 128-chunks for 2C
W = 128
H = 128
HW = H * W
NB = 2048
NBLK = HW // NB
RB = NB // W      # rows per block (16)
EXT = NB + 2 * W  # ext columns (with halo)
GOFF = 2          # left zero-guard cols (even => 4B-aligned dx=0 taps)
GEXT = EXT + 2 * GOFF
SPB = 6           # segments per block
NSEG = NBLK * SPB  # 48
# segment boundaries within a block (pattern repeats every block)
SEGB = [0, 342, 683, 1024, 1366, 1707, 2048]

TAPS = [(dy, dx) for dy in (-1, 0, 1) for dx in (-1, 0, 1)]
CENTER = 4


def head_pieces():
    """Per head: list of (mc, p0, p1, s0): global channels
    [mc*128+p0, mc*128+p1) == within-head channels [s0, s0+(p1-p0))."""
    out = []
    for h in range(HEADS):
        c0, c1 = h * HD, (h + 1) * HD
        pieces = []
        c = c0
        while c < c1:
            mc = c // 128
            p0 = c - mc * 128
            p1 = min(128, c1 - mc * 128)
            pieces.append((mc, p0, p1, c - c0))
            c = mc * 128 + p1
        out.append(pieces)
    return out


def build(cfg, timing_reps=0):
    """cfg keys:
      dw_modes: list of 6 entries 'd' | 'h' | 's'   (v-ocs may be 'd')
      pool_blocks: dict oc -> set of blk indices whose chain runs on Pool
      evac_dve: fraction knobs: (kv_dve_mod, y_dve_mod) alternation mods
    """
    pe_blocks = {k: set(v) for k, v in (cfg.get("pe_blocks") or {}).items()}
    pool_blocks = {k: set(v) for k, v in (cfg.get("pool_blocks") or {}).items()}
    kv_pat = cfg.get("kv_pat", "A")   # evac engine per chunk: A/D
    y_pat = cfg.get("y_pat", "A")
    dw_pat = cfg.get("dw_pat", "A")
    timing = timing_reps > 0

    nc = bacc.Bacc("TRN2", target_bir_lowering=False)

    # ---- DRAM parameters ----
    if timing:
        tok_d = nc.declare_dram_parameter("tok", [1, 1], f32, isOutput=False)
        toko_d = nc.declare_dram_parameter("tok_out", [1, 1], f32, isOutput=True)
        x_d = nc.dram_tensor("x", [C, HW], bf16)
        y_d = nc.dram_tensor("y", [C, HW], bf16)
    else:
        x_d = nc.declare_dram_parameter("x", [C, HW], bf16, isOutput=False)
        y_d = nc.declare_dram_parameter("y", [C, HW], bf16, isOutput=True)
    wkv_d = nc.declare_dram_parameter("wkv", [128, CC, C2], bf16, isOutput=False)
    dws_d = nc.declare_dram_parameter("dws", [128, OC, 9], f32, isOutput=False)
    dwsn_d = nc.declare_dram_parameter("dwsn", [128, OC, 9], f32, isOutput=False)
    kvb_d = nc.declare_dram_parameter("kvb", [128, OC], f32, isOutput=False)
    dwb_d = nc.declare_dram_parameter("dwb", [128, OC], f32, isOutput=False)

    diag_ocs = [oc for oc in range(OC) if pe_blocks.get(oc)]
    diag_slots = {}
    nslot = 0
    for oc in diag_ocs:
        diag_slots[oc] = {t: nslot + t for t in range(9)}
        nslot += 9
    if nslot:
        diag_d = nc.declare_dram_parameter("diag", [128, nslot, 128], bf16,
                                           isOutput=False)
    projT_d = nc.declare_dram_parameter("projT", [HD, HEADS, C], bf16, isOutput=False)
    projb_d = nc.declare_dram_parameter("projb", [128, CC], f32, isOutput=False)
    tempP_d = nc.declare_dram_parameter("tempP", [128, CC], f32, isOutput=False)
    q48T_d = nc.declare_dram_parameter("q48T", [HD, HEADS, HD], bf16, isOutput=False)
    ident_d = nc.declare_dram_parameter("ident", [128, 128], bf16, isOutput=False)

    debug = cfg.get("debug", False)
    if debug:
        dbg_ksum = nc.declare_dram_parameter("dbg_ksum", [128, CC, NBLK, SPB], f32, isOutput=True)
        dbg_rn = nc.declare_dram_parameter("dbg_rn", [128, CC], f32, isOutput=True)
        dbg_lg = nc.declare_dram_parameter("dbg_lg", [HD, HEADS, HD], f32, isOutput=True)
        dbg_mbt = nc.declare_dram_parameter("dbg_mbt", [128, CC, C], f32, isOutput=True)
        dbg_n2 = nc.declare_dram_parameter("dbg_n2", [128, CC], f32, isOutput=True)
        dbg_mbs = nc.declare_dram_parameter("dbg_mbs", [128, CC, C], f32, isOutput=True)

    v12_dram = nc.dram_tensor("v12_spill", [2, 128, HW], bf16)

    xv = x_d[:, :].rearrange("(cc p) n -> p cc n", p=128)
    yv = y_d[:, :].rearrange("(cc p) n -> p cc n", p=128)

    pieces = head_pieces()

    with tile.TileContext(nc) as tc, ExitStack() as ctx:
        const = ctx.enter_context(tc.tile_pool(name="const", bufs=1))
        wkv = const.tile([128, CC, C2], bf16)
        nc.sync.dma_start(out=wkv, in_=wkv_d[:, :, :])
        dws = const.tile([128, OC, 9], f32)
        nc.sync.dma_start(out=dws, in_=dws_d[:, :, :])
        dwsn = const.tile([128, OC, 9], f32)
        nc.sync.dma_start(out=dwsn, in_=dwsn_d[:, :, :])
        kvb = const.tile([128, OC], f32)
        nc.sync.dma_start(out=kvb, in_=kvb_d[:, :])
        dwb = const.tile([128, OC], f32)
        nc.sync.dma_start(out=dwb, in_=dwb_d[:, :])
        if nslot:
            diag = const.tile([128, nslot, 128], bf16)
            nc.sync.dma_start(out=diag, in_=diag_d[:, :, :])
        projT = const.tile([HD, HEADS, C], bf16)
        nc.sync.dma_start(out=projT, in_=projT_d[:, :, :])
        projb = const.tile([128, CC], f32)
        nc.sync.dma_start(out=projb, in_=projb_d[:, :])
        tempP = const.tile([128, CC], f32)
        nc.sync.dma_start(out=tempP, in_=tempP_d[:, :])
        q48T = const.tile([HD, HEADS, HD], bf16)
        nc.sync.dma_start(out=q48T, in_=q48T_d[:, :, :])
        ident = const.tile([128, 128], bf16)
        nc.sync.dma_start(out=ident, in_=ident_d[:, :])

        # persistent state
        v0s = const.tile([128, HW], bf16)               # v for cc 0
        ksum = const.tile([128, CC, NBLK, SPB], f32)
        tmps_po = const.tile([128, 3, CC, NBLK, SPB], f32)
        statsT = const.tile([128, CC, NBLK, SPB, 6], f32)
        MbT = const.tile([128, CC, C], bf16)

        if timing:
            tokt = const.tile([1, 1], f32)
            nc.sync.dma_start(out=tokt, in_=tok_d[:, :])
            nc.sync.dma_start(out=toko_d[:, :], in_=tokt)
            loop_cm = tc.For_i(0, timing_reps, 1)
        else:
            loop_cm = contextlib.nullcontext()

        with loop_cm, ExitStack() as lctx:
            p1 = lctx.enter_context(ExitStack())
            xext = p1.enter_context(tc.tile_pool(name="xext", bufs=2))
            kv0p = p1.enter_context(tc.tile_pool(name="kv0", bufs=2))
            kscrp = p1.enter_context(tc.tile_pool(name="kscr", bufs=2))
            v2sp = p1.enter_context(tc.tile_pool(name="v2s", bufs=2))
            kvmp = p1.enter_context(tc.tile_pool(name="kvm", bufs=1))
            tmpp = p1.enter_context(tc.tile_pool(name="tmp", bufs=3))
            psc = p1.enter_context(tc.tile_pool(name="psc", bufs=2, space="PSUM"))
            psd = p1.enter_context(tc.tile_pool(name="psd", bufs=2, space="PSUM"))

            def dw_pe_taps(oc, kv0, ps, cs):
                """9 taps via PE diag matmuls into psum chunk ps covering
                block cols [cs, cs+1024); tap-outer within the chunk."""
                slots = diag_slots[oc]
                for t, (dy, dx) in enumerate(TAPS):
                    base = GOFF + W + cs + dy * W + dx
                    for h0 in (0, 512):
                        nc.tensor.matmul(
                            ps[:, h0:h0 + 512], lhsT=diag[:, slots[t], :],
                            rhs=kv0[:, 0, base + h0:base + h0 + 512],
                            start=(t == 0), stop=(t == 8))

            def wrap_fixups(oc, kv0, dst, eng):
                """Subtract row-wrap contamination of flat dx-taps at w=0 /
                w=W-1 columns of the SBUF dst."""
                for t, (dy, dx) in enumerate(TAPS):
                    if dx == 0:
                        continue
                    sc = dwsn[:, oc, t:t + 1]
                    if dx == 1:
                        srcc = kv0[:, 0, GOFF + (dy + 2) * W::W][:, :RB]
                        dstc = dst[:, W - 1::W]
                    else:
                        srcc = kv0[:, 0, GOFF - 1 + (dy + 1) * W::W][:, :RB]
                        dstc = dst[:, 0::W]
                    eng.scalar_tensor_tensor(
                        dstc, srcc, sc, dstc, ALU.mult, ALU.add)

            def dw_chain(oc, kv0, dst, tt_pool):
                """dwconv as TS multiplies (DVE 4x via aligned kvm copy) +
                TT adds on DVE (2x) or Pool.  STT is 1x-only on this DVE --
                TS+TT pairs are strictly faster."""
                kvm = kvmp.tile([128, GEXT], bf16, tag="kvm")
                nc.vector.tensor_copy(kvm[:, 0:GEXT - 2], kv0[:, 0, 1:GEXT - 1])
                nc.vector.tensor_scalar(
                    dst, kv0[:, 0, GOFF + W:GOFF + W + NB],
                    dws[:, oc, CENTER:CENTER + 1], dwb[:, oc:oc + 1],
                    ALU.mult, ALU.add)
                tte = nc.gpsimd if tt_pool else nc.vector
                for t, (dy, dx) in enumerate(TAPS):
                    if t == CENTER:
                        continue
                    sc = dws[:, oc, t:t + 1]
                    if dx == 0:
                        srcf = kv0[:, 0, GOFF + (1 + dy) * W:
                                   GOFF + (1 + dy) * W + NB]
                    elif dx == 1:
                        srcf = kvm[:, GOFF + (1 + dy) * W:
                                   GOFF + (1 + dy) * W + NB]
                    else:
                        srcf = kvm[:, GOFF + (1 + dy) * W - 2:
                                   GOFF + (1 + dy) * W - 2 + NB]
                    tmp = tmpp.tile([128, NB], bf16, tag="tmp")
                    nc.vector.tensor_scalar(tmp, srcf, sc, None, ALU.mult)
                    tte.tensor_tensor(dst, dst, tmp, ALU.add)
                wrap_fixups(oc, kv0, dst, nc.vector)

            dwecnt = [0]

            def dw_diag(oc, kv0, dst):
                """All 9 taps on PE diag matmuls; ACT/DVE evac (+bias) to
                SBUF dst; cheap SBUF wrap fixups on DVE."""
                for nch in range(NB // 1024):
                    cs = nch * 1024
                    ps = psd.tile([128, 1024], f32, tag="psd")
                    dw_pe_taps(oc, kv0, ps, cs)
                    dsl = dst[:, cs:cs + 1024]
                    e = dw_pat[dwecnt[0] % len(dw_pat)]
                    dwecnt[0] += 1
                    if e == "A":
                        nc.scalar.activation(dsl, ps, AF.Identity,
                                             bias=dwb[:, oc:oc + 1])
                    else:
                        nc.vector.tensor_scalar_add(dsl, ps, dwb[:, oc:oc + 1])
                wrap_fixups(oc, kv0, dst, nc.vector)

            # flat (blk, oc) pipeline: kvconv of step j overlaps dwconv of
            # step j-1 so PE never waits on ACT's kv0 evacuation.
            OCORD = [3, 4, 5, 0, 1, 2]
            cgrp = [(0, 1024), (1024, 1024), (2048, 256)]
            ecnt = 0
            steps = [(blk, oc) for blk in range(NBLK) for oc in OCORD]
            pend = None
            kdsts = {}
            vsts = {}

            def emit_kvconv(blk, oc):
                nonlocal ecnt
                n0 = blk * NB
                lo = max(0, n0 - W)
                hi = min(HW, n0 + NB + W)
                xe = xes[blk]
                kv0 = kv0p.tile([128, 1, GEXT], bf16, tag=f"kv0_{oc}",
                                name=f"kv0_{oc}")
                for (cs, cl) in cgrp:
                    ps = psc.tile([128, 1024], f32, tag="psc")
                    for cc in range(CC):
                        for h0 in range(0, cl, 512):
                            hl = min(512, cl - h0)
                            nc.tensor.matmul(
                                ps[:, h0:h0 + hl],
                                lhsT=wkv[:, cc, oc * 128:(oc + 1) * 128],
                                rhs=xe[:, cc, cs + h0:cs + h0 + hl],
                                start=(cc == 0), stop=(cc == CC - 1))
                    dst = kv0[:, 0, GOFF + cs:GOFF + cs + cl]
                    e = kv_pat[ecnt % len(kv_pat)]
                    ecnt += 1
                    if e == "D":
                        nc.vector.tensor_scalar_add(dst, ps[:, :cl],
                                                    kvb[:, oc:oc + 1])
                    else:
                        nc.scalar.activation(dst, ps[:, :cl], AF.Identity,
                                             bias=kvb[:, oc:oc + 1])
                if lo == 0:
                    nc.gpsimd.memset(kv0[:, :, :GOFF + W], 0.0)
                else:
                    nc.gpsimd.memset(kv0[:, :, 0:GOFF], 0.0)
                if hi == HW:
                    nc.gpsimd.memset(kv0[:, :, GOFF + W + NB:], 0.0)
                else:
                    nc.gpsimd.memset(kv0[:, :, GEXT - GOFF:], 0.0)
                return kv0

            def emit_dwconv(blk, oc, kv0):
                n0 = blk * NB
                if oc < CC:
                    dst = kscrp.tile([128, NB], bf16, tag=f"k{oc}",
                                     name=f"kscr{oc}")
                    kdsts.setdefault(blk, {})[oc] = dst
                elif oc == CC:
                    dst = v0s[:, n0:n0 + NB]
                else:
                    dst = v2sp.tile([128, NB], bf16, tag=f"v{oc}s",
                                    name=f"vst{oc}")
                    vsts[(blk, oc)] = dst
                if blk in pe_blocks.get(oc, ()):
                    dw_diag(oc, kv0, dst)
                else:
                    dw_chain(oc, kv0, dst,
                             tt_pool=blk in pool_blocks.get(oc, ()))
                if (blk, oc) in vsts:
                    nc.sync.dma_start(
                        out=v12_dram[oc - CC - 1, :, n0:n0 + NB],
                        in_=vsts.pop((blk, oc)))
                if oc == OCORD[-1]:
                    emit_stats(blk)

            def emit_stats(blk):
                kdst = kdsts.pop(blk)
                for cc in range(CC):
                    for s in range(SPB):
                        nc.vector.bn_stats(
                            statsT[:, cc, blk, s, :],
                            kdst[cc][:, SEGB[s]:SEGB[s + 1]])
                stb = statsT[:, :, blk, :, :]
                t1b = tmps_po[:, 0, :, blk, :]
                t2b = tmps_po[:, 1, :, blk, :]
                ksb_sl = ksum[:, :, blk, :]
                nc.vector.tensor_tensor(t1b, stb[:, :, :, 0], stb[:, :, :, 1], ALU.mult)
                nc.vector.tensor_tensor(t2b, stb[:, :, :, 3], stb[:, :, :, 4], ALU.mult)
                nc.vector.tensor_tensor(ksb_sl, t1b, t2b, ALU.add)
                u1 = tmps_po[:, 2, :, blk, :]
                nc.vector.tensor_tensor(u1, t1b, stb[:, :, :, 1], ALU.mult)
                nc.vector.tensor_tensor(t2b, t2b, stb[:, :, :, 4], ALU.mult)
                nc.vector.tensor_tensor(u1, u1, t2b, ALU.add)
                nc.vector.tensor_tensor(u1, u1, stb[:, :, :, 2], ALU.add)
                nc.vector.tensor_tensor(u1, u1, stb[:, :, :, 5], ALU.add)

            xes = {}

            def load_xe(blk):
                n0 = blk * NB
                lo = max(0, n0 - W)
                hi = min(HW, n0 + NB + W)
                off = lo - (n0 - W)
                xe = xext.tile([128, CC, EXT], bf16)
                if off > 0:
                    nc.vector.memset(xe[:, :, :off], 0.0)
                if (n0 - W) + EXT > hi:
                    nc.vector.memset(xe[:, :, off + (hi - lo):], 0.0)
                nc.gpsimd.dma_start(out=xe[:, :, off:off + (hi - lo)],
                                    in_=xv[:, :, lo:hi])
                xes[blk] = xe

            lag = cfg.get("lag", 1)
            load_xe(0)
            load_xe(1)
            for j, (blk, oc) in enumerate(steps):
                kv0 = emit_kvconv(blk, oc)
                if oc == OCORD[0] and blk + 2 < NBLK:
                    load_xe(blk + 2)
                if lag == 0:
                    emit_dwconv(blk, oc, kv0)
                else:
                    if pend is not None:
                        emit_dwconv(*pend)
                    pend = (blk, oc, kv0)
            if lag and pend is not None:
                emit_dwconv(*pend)

            p1.close()

            # ---------- middle ----------
            pmid = lctx.enter_context(ExitStack())
            mid = pmid.enter_context(tc.tile_pool(name="mid", bufs=1))
            psm = pmid.enter_context(tc.tile_pool(name="psm", bufs=1, space="PSUM"))
            pst = pmid.enter_context(tc.tile_pool(name="pst", bufs=1, space="PSUM"))

            n2 = mid.tile([128, CC], f32)
            nc.vector.tensor_reduce(
                n2, tmps_po[:, 2, :, :, :], axis=AX.XY, op=ALU.add)
            # rnorm = temp / max(sqrt(n2), eps), one Newton step
            sn = mid.tile([128, CC], f32)
            nc.scalar.activation(sn, n2, AF.Sqrt)
            nc.vector.tensor_scalar_max(sn, sn, 1e-12)
            rn = mid.tile([128, CC], f32)
            nc.vector.reciprocal(rn, sn)
            t1 = mid.tile([128, CC], f32)
            nc.vector.tensor_tensor(t1, rn, rn, ALU.mult)
            nc.vector.tensor_tensor(t1, t1, n2, ALU.mult)
            nc.vector.tensor_scalar(t1, t1, -0.5, 1.5, ALU.mult, ALU.add)
            nc.vector.tensor_tensor(rn, rn, t1, ALU.mult)
            nc.vector.tensor_tensor(rn, rn, tempP, ALU.mult)
            # ksb = bf16(rn * ksum)
            ksb = mid.tile([128, CC, NSEG], bf16)
            for cc in range(CC):
                nc.vector.tensor_scalar(
                    ksb[:, cc, :],
                    ksum[:, cc, :, :].rearrange("p b s -> p (b s)"),
                    rn[:, cc:cc + 1], None, ALU.mult)
            # KsT[j, d] via PE transpose
            psT = pst.tile([48, CC, 128], bf16)
            for cc in range(CC):
                nc.tensor.transpose(psT[:, cc, :], ksb[:, cc, :], ident)
            KsT = mid.tile([48, CC, 128], bf16)
            nc.vector.tensor_copy(KsT, psT)
            # lg[c, h, d] = sum_j q48T[j, h, c] * KsT[j, d]
            lgp = pst.tile([HD, HEADS, HD], f32, tag="lgp")
            for h in range(HEADS):
                for (mc, p0, p1_, s0) in pieces[h]:
                    nc.tensor.matmul(
                        lgp[:, h, s0:s0 + (p1_ - p0)],
                        lhsT=q48T[:, h, :], rhs=KsT[:, mc, p0:p1_],
                        start=True, stop=True)
            lg = mid.tile([HD, HEADS, HD], f32)
            nc.vector.tensor_copy(lg, lgp)
            lg2 = lg.rearrange("p h d -> p (h d)")
            mx = mid.tile([HD, HEADS], f32)
            nc.vector.tensor_reduce(mx, lg, axis=AX.X, op=ALU.max)
            nc.vector.tensor_tensor(
                lg, lg, mx[:, :, None].broadcast_to([HD, HEADS, HD]), ALU.subtract)
            nc.scalar.activation(lg2, lg2, AF.Exp)
            sm = mid.tile([HD, HEADS], f32)
            nc.vector.tensor_reduce(sm, lg, axis=AX.X, op=ALU.add)
            nc.vector.reciprocal(sm, sm)
            nc.vector.tensor_tensor(
                lg, lg, sm[:, :, None].broadcast_to([HD, HEADS, HD]), ALU.mult)
            attnb = mid.tile([HD, HEADS, HD], bf16)
            nc.vector.tensor_copy(attnb, lg)

            # Mb[co, d] per mc chunk: pmb[:, mc, h, :] = projT[:,h,mc]^T @ attn
            # one PSUM bank (512 f32) per mc so no (mc, h) slice straddles
            pmb = psm.tile([128, CC, 512], f32)
            for h in range(HEADS):
                for mc in range(CC):
                    nc.tensor.matmul(pmb[:, mc, h * HD:(h + 1) * HD],
                                     lhsT=projT[:, h, mc * 128:(mc + 1) * 128],
                                     rhs=attnb[:, h, :], start=True, stop=True)
            Mb_s = mid.tile([128, CC, C], bf16)
            nc.scalar.activation(Mb_s, pmb[:, :, :C], AF.Identity)
            psMbT = psm.tile([128, CC, C], bf16, tag="psMbT", name="psMbT")
            for dc in range(CC):
                for mc in range(CC):
                    nc.tensor.transpose(
                        psMbT[:, dc, mc * 128:(mc + 1) * 128],
                        Mb_s[:, mc, dc * 128:(dc + 1) * 128], ident)
            nc.vector.tensor_copy(MbT, psMbT)
            if debug:
                nc.sync.dma_start(out=dbg_ksum[:, :, :, :], in_=ksum)
                nc.sync.dma_start(out=dbg_rn[:, :], in_=rn)
                dbg_lg_t = mid.tile([HD, HEADS, HD], f32)
                nc.vector.tensor_copy(dbg_lg_t, lg)
                nc.sync.dma_start(out=dbg_lg[:, :, :], in_=dbg_lg_t)
                dbg_mbt_t = mid.tile([128, CC, C], f32)
                nc.vector.tensor_copy(dbg_mbt_t, MbT)
                nc.sync.dma_start(out=dbg_mbt[:, :, :], in_=dbg_mbt_t)
                nc.sync.dma_start(out=dbg_n2[:, :], in_=n2)
                dbg_mbs_t = mid.tile([128, CC, C], f32)
                nc.vector.tensor_copy(dbg_mbs_t, Mb_s)
                nc.sync.dma_start(out=dbg_mbs[:, :, :], in_=dbg_mbs_t)

            pmid.close()

            # ---------- pass 2 ----------
            p2 = lctx.enter_context(ExitStack())
            vbp = p2.enter_context(tc.tile_pool(name="vb2", bufs=2))
            outp = p2.enter_context(tc.tile_pool(name="outp", bufs=3))
            psf = p2.enter_context(tc.tile_pool(name="psf", bufs=2, space="PSUM"))
            yecnt = 0
            for blk in range(NBLK):
                n0 = blk * NB
                vb12 = vbp.tile([128, 2, NB], bf16, tag="vb12", name="vb12")
                nc.sync.dma_start(
                    out=vb12,
                    in_=v12_dram[:, :, n0:n0 + NB].rearrange("s p n -> p s n"))
                for oc in range(CC):
                    ot = outp.tile([128, NB], bf16)
                    ps = psf.tile([128, NB], f32)
                    for dc in range(CC):
                        vsrc = (v0s[:, n0:n0 + NB] if dc == 0
                                else vb12[:, dc - 1, :])
                        for h0 in range(0, NB, 512):
                            nc.tensor.matmul(
                                ps[:, h0:h0 + 512],
                                lhsT=MbT[:, dc, oc * 128:(oc + 1) * 128],
                                rhs=vsrc[:, h0:h0 + 512],
                                start=(dc == 0), stop=(dc == CC - 1))
                    e = y_pat[yecnt % len(y_pat)]
                    yecnt += 1
                    if e == "D":
                        nc.vector.tensor_scalar_add(ot, ps, projb[:, oc:oc + 1])
                    else:
                        nc.scalar.activation(ot, ps, AF.Identity,
                                             bias=projb[:, oc:oc + 1])
                    nc.sync.dma_start(out=yv[:, oc, n0:n0 + NB], in_=ot)
            p2.close()

    nc.compile()
    return nc


def host_prep(inputs, cfg):
    """Full inputs (numpy, reference layout) -> per-core in_maps list."""
    x = np.ascontiguousarray(inputs["x"]).reshape(-1, C, HW)
    B = x.shape[0]
    qp = np.asarray(inputs["q_param"])[0]              # [heads, hd, 48]
    temp = np.asarray(inputs["temperature"]).reshape(HEADS)
    kv_w = np.asarray(inputs["kv_w"])[:, :, 0, 0]      # [768, 384]
    kv_b = np.asarray(inputs["kv_b"])
    dw_w = np.asarray(inputs["dw_w"])[:, 0]            # [768, 3, 3]
    dw_b = np.asarray(inputs["dw_b"])
    pw = np.asarray(inputs["proj_w"])[:, :, 0, 0]      # [384, 384]
    pb = np.asarray(inputs["proj_b"])

    wkv = np.ascontiguousarray(
        kv_w.T.reshape(CC, 128, C2).transpose(1, 0, 2)).astype(ml_dtypes.bfloat16)
    dws = np.ascontiguousarray(
        dw_w.reshape(OC, 128, 9).transpose(1, 0, 2)).astype(np.float32)
    kvb = np.ascontiguousarray(kv_b.reshape(OC, 128).T).astype(np.float32)
    dwb = np.ascontiguousarray(dw_b.reshape(OC, 128).T).astype(np.float32)

    pe_blocks = cfg.get("pe_blocks") or {}
    slot_list = []
    for oc in range(OC):
        if not pe_blocks.get(oc):
            continue
        for t in range(9):
            slot_list.append((oc, t))
    diag = np.zeros((128, max(len(slot_list), 1), 128), np.float32)
    for i, (oc, t) in enumerate(slot_list):
        s = dw_w[oc * 128:(oc + 1) * 128, t // 3, t % 3]
        diag[np.arange(128), i, np.arange(128)] = s
    diag = diag.astype(ml_dtypes.bfloat16)

    projT = np.ascontiguousarray(
        pw.T.reshape(HEADS, HD, C).transpose(1, 0, 2)).astype(ml_dtypes.bfloat16)
    projb = np.ascontiguousarray(pb.reshape(CC, 128).T).astype(np.float32)
    tempP = np.zeros((128, CC), np.float32)
    for cc in range(CC):
        for p in range(128):
            tempP[p, cc] = temp[(cc * 128 + p) // HD]
    # q48T[j, h, c] = qp[h, c, j]
    q48T = np.ascontiguousarray(qp.transpose(2, 0, 1)).astype(ml_dtypes.bfloat16)
    ident = np.eye(128, dtype=np.float32).astype(ml_dtypes.bfloat16)

    shared = dict(wkv=wkv, dws=dws, dwsn=(-dws).astype(np.float32),
                  kvb=kvb, dwb=dwb, projT=projT, projb=projb, tempP=tempP,
                  q48T=q48T, ident=ident)
    if slot_list:
        shared["diag"] = diag
    in_maps = []
    for b in range(B):
        m = dict(shared)
        m["x"] = x[b].astype(ml_dtypes.bfloat16)
        in_maps.append(m)
    return in_maps


# ---------------------------------------------------------------------------
# Harness entry point: kernel(**inputs) -> full output (B, C, H, W) float32.
# ---------------------------------------------------------------------------

def _carve(ndve, npool):
    pe = {oc: set(range(8)) for oc in range(6)}
    pool = {}
    blkorder = [0, 2, 4, 1, 3, 5, 6, 7]
    rr = []
    for depth, oc in enumerate((5, 4, 3, 2, 1, 0)):
        for b in blkorder:
            rr.append((oc, b))
    for oc, b in rr[:ndve]:
        pe[oc].discard(b)
    for oc, b in rr[ndve:ndve + npool]:
        pe[oc].discard(b)
        pool.setdefault(oc, set()).add(b)
    return pe, pool


_PE_BLOCKS, _POOL_BLOCKS = _carve(18, 0)
CFG = dict(
    pe_blocks=_PE_BLOCKS,
    pool_blocks=_POOL_BLOCKS,
    kv_pat="A",
    y_pat="A",
    lag=0,
)

_PROGRAM_CACHE = {}


def _get_program():
    key = "main"
    if key not in _PROGRAM_CACHE:
        _PROGRAM_CACHE[key] = build(CFG)
    return _PROGRAM_CACHE[key]


def kernel(**inputs):
    from concourse.bass_utils import run_bass_kernel_spmd

    x = np.asarray(inputs["x"])
    B, Cin, H_, W_ = x.shape
    assert (Cin, H_, W_) == (C, H, W) and B == 8
    nc = _get_program()
    in_maps = host_prep(inputs, CFG)
    res = run_bass_kernel_spmd(nc, in_maps, list(range(8)))
    out = np.stack([np.asarray(res.results[b]["y"]).astype(np.float32)
                    .reshape(C, H_, W_) for b in range(B)])
    return out
